# revision 5
# baseline (speedup 1.0000x reference)
"""Trainium2 Bass kernel for nn_BerTII (masked-mean embedding bag -> 1-dim
linear -> sigmoid), distributed over 8 NeuronCores.

reference math:
  mask[b,l] = l < lengths[b]
  pooled[b,:] = sum_l mask[b,l] * emb[tokens[b,l],:] / L
  out[b] = sigmoid(pooled[b,:] @ W.T + bias)

The 1-output linear commutes with the masked mean:
  out[b] = sigmoid( (1/L) * sum_{l<len_b} (emb[tokens[b,l]] . W) + bias )
so the kernel never materializes the [B,L,P] gather. Host-side marshaling is
integer-only index work (the "all-to-all" of the sharding hint done at
input-staging time):
  - flatten all valid (b,l) tokens, dedupe globally (np.unique) and build a
    per-(unique-row, batch) multiplicity matrix;
  - split the unique rows into 8 equal-count contiguous chunks; core c
    receives ONLY the vocab slice spanning its chunk (rebased int16 indices),
    so the 800MB table is sharded across cores, not replicated; rows are
    padded 1000->1024 floats so each row is one 4KB 256B-aligned gather
    element;
  - each core bulk-gathers its ~U/8 rows with InstDMAGatherAnt, dots each row
    with W on the Vector engine (scalar_tensor_tensor accum), and accumulates
    per-batch partial dot products with tiny PE matmuls against the
    multiplicity matrix (y stationary [128,1], counts moving [128,64]);
  - an 8-core AllReduce(add) of the [64] partials, then sigmoid(x/L + b) on
    the Scalar engine. Every core emits the full [64] output; core 0's is
    returned.
"""
import os
import sys

sys.path.insert(0, "/opt/trn_rl_repo")

import numpy as np

VOCAB = 200000
PDIM = 1000
PDIMP = 1024  # row stride padded to 256B multiple for dma_gather
B = 64
L = 2048
NCORES = 8

LAST = {}  # debug: last BassKernelResults etc.


# ---------------------------------------------------------------------------
# walrus legalization: this toolchain allows at most ONE semaphore wait per
# instruction ("Too many sync wait commands"); split extras onto NoOps.
def _legalize_sem_waits(nc, mybir, max_waits=1):
    n = 0
    for f in nc.m.functions:
        for bb in f.blocks:
            new = []
            for inst in bb.instructions:
                si = inst.sync_info
                if si is not None and si.on_wait and len(si.on_wait) > max_waits:
                    waits = list(si.on_wait)
                    extra, keep = waits[:-max_waits], waits[-max_waits:]
                    k = 0
                    while extra:
                        chunk, extra = extra[:max_waits], extra[max_waits:]
                        new.append(
                            mybir.InstNoOp(
                                name=f"{inst.name}-ws{k}",
                                sync_info=mybir.SyncInfo(on_wait=chunk, on_update=[]),
                                bass_nofuse=True,
                                engine=inst.engine,
                            )
                        )
                        k += 1
                        n += 1
                    si.on_wait = keep
                new.append(inst)
            bb.instructions[:] = new
    return n


def _build(Vmax, T, chunk, gbufs, mode="dmag", legalize=True):
    from concourse import bass, bacc, mybir
    import concourse.tile as tile

    F32 = mybir.dt.float32
    I16 = mybir.dt.int16
    I32 = mybir.dt.int32

    nc = bacc.Bacc(None, num_devices=NCORES)
    emb = nc.declare_dram_parameter("emb", [Vmax, PDIMP], F32, isOutput=False)
    # idx16: gather index i of this core lives at [i % 16, i // 16], rows
    # replicated x8 down the partition dim (one copy per Q7 band).
    idx16 = nc.declare_dram_parameter("idx16", [128, T * 8], I16, isOutput=False)
    idx32 = nc.declare_dram_parameter("idx32", [128, T], I32, isOutput=False)
    sel = nc.declare_dram_parameter("sel", [128, T * B], F32, isOutput=False)
    wrep = nc.declare_dram_parameter("wrep", [128, PDIM], F32, isOutput=False)
    brep = nc.declare_dram_parameter("brep", [1, 1], F32, isOutput=False)
    outp = nc.declare_dram_parameter("out", [1, B], F32, isOutput=True)

    with tile.TileContext(nc) as tc:
        with (
            tc.tile_pool(name="meta", bufs=1) as meta,
            tc.tile_pool(name="g", bufs=gbufs) as gp,
            tc.tile_pool(name="y", bufs=4) as yp,
            tc.tile_pool(name="ps", bufs=1, space="PSUM") as pp,
            tc.tile_pool(name="dram", bufs=1, space="DRAM") as dp,
        ):
            idx16_sb = meta.tile([128, T * 8], I16)
            nc.sync.dma_start(out=idx16_sb[:], in_=idx16[:])
            idx32_sb = meta.tile([128, T], I32)
            nc.sync.dma_start(out=idx32_sb[:], in_=idx32[:])
            sel_sb = meta.tile([128, T * B], F32)
            nc.sync.dma_start(out=sel_sb[:], in_=sel[:])
            w_sb = meta.tile([128, PDIM], F32)
            nc.sync.dma_start(out=w_sb[:], in_=wrep[:])
            b_sb = meta.tile([1, 1], F32)
            nc.sync.dma_start(out=b_sb[:], in_=brep[:])

            dot_ps = pp.tile([1, B], F32)

            def consume(gflat, off, t):
                """gflat: [128, >=off+PDIM] gathered rows tile; tile index t."""
                y = yp.tile([128, 1], F32)
                gs = gflat[:, off : off + PDIM]
                nc.vector.scalar_tensor_tensor(
                    out=gs,
                    in0=gs,
                    scalar=1.0,
                    in1=w_sb[:],
                    op0=mybir.AluOpType.mult,
                    op1=mybir.AluOpType.mult,
                    accum_out=y[:],
                )
                nc.tensor.matmul(
                    out=dot_ps[:],
                    lhsT=y[:],
                    rhs=sel_sb[:, t * B : (t + 1) * B],
                    start=(t == 0),
                    stop=(t == T - 1),
                )

            if mode == "dmag":
                s = 0
                while s < T:
                    c = min(chunk, T - s)
                    g = gp.tile([128, c, PDIMP], F32, tag="g")
                    nc.gpsimd.dma_gather(
                        out_ap=g[:],
                        in_ap=emb[:],
                        idxs_ap=idx16_sb[:, s * 8 : (s + c) * 8],
                        num_idxs=c * 128,
                        num_idxs_reg=c * 128,
                        elem_size=PDIMP,
                    )
                    gflat = g[:].rearrange("p c e -> p (c e)")
                    for j in range(c):
                        consume(gflat, j * PDIMP, s + j)
                    s += c
            else:  # indirect: one [128, PDIMP] row-gather per tile
                for t in range(T):
                    g = gp.tile([128, PDIMP], F32, tag="g")
                    nc.gpsimd.indirect_dma_start(
                        out=g[:],
                        out_offset=None,
                        in_=emb[:],
                        in_offset=bass.IndirectOffsetOnAxis(
                            ap=idx32_sb[:, t : t + 1], axis=0
                        ),
                    )
                    consume(g[:], 0, t)

            part_sb = meta.tile([1, B], F32)
            nc.vector.tensor_copy(out=part_sb[:], in_=dot_ps[:])
            cc_in = dp.tile([1, B], F32)
            cc_out = dp.tile([1, B], F32)
            nc.sync.dma_start(out=cc_in[:], in_=part_sb[:])
            nc.gpsimd.collective_compute(
                "AllReduce",
                mybir.AluOpType.add,
                replica_groups=[list(range(NCORES))],
                ins=[cc_in[:]],
                outs=[cc_out[:]],
            )
            red_sb = meta.tile([1, B], F32)
            nc.sync.dma_start(out=red_sb[:], in_=cc_out[:])
            o_sb = meta.tile([1, B], F32)
            nc.scalar.activation(
                out=o_sb[:],
                in_=red_sb[:],
                func=mybir.ActivationFunctionType.Sigmoid,
                bias=b_sb[:],
                scale=1.0 / float(L),
            )
            nc.sync.dma_start(out=outp[:], in_=o_sb[:])

    nc.compile()
    if legalize:
        _legalize_sem_waits(nc, mybir)
    return nc


def _marshal(tokens, lengths, emb_table, W, b):
    tokens = np.asarray(tokens)
    lengths = np.asarray(lengths).astype(np.int64)
    emb_table = np.ascontiguousarray(emb_table, dtype=np.float32)

    mask = np.arange(L)[None, :] < lengths[:, None]
    flat_tok = tokens[mask].astype(np.int64)
    flat_b = np.broadcast_to(np.arange(B)[:, None], (B, L))[mask]
    uniq, inv = np.unique(flat_tok, return_inverse=True)
    U = len(uniq)
    cnt = np.zeros((U, B), dtype=np.float32)
    np.add.at(cnt, (inv, flat_b), 1.0)

    bounds = [U * c // NCORES for c in range(NCORES + 1)]
    rows_max = max(bounds[c + 1] - bounds[c] for c in range(NCORES))
    T = -(-rows_max // 128)

    spans = []
    for c in range(NCORES):
        s, e = bounds[c], bounds[c + 1]
        lo = int(uniq[s]) if e > s else 0
        hi = int(uniq[e - 1]) + 1 if e > s else 1
        spans.append((s, e, lo, hi))
    Vmax = max(hi - lo for _, _, lo, hi in spans)

    wrep = np.broadcast_to(
        np.asarray(W, dtype=np.float32).reshape(1, PDIM), (128, PDIM)
    ).copy()
    brep = np.full((1, 1), np.float32(np.asarray(b).reshape(-1)[0]), dtype=np.float32)

    in_maps = []
    for c in range(NCORES):
        s, e, lo, hi = spans[c]
        span = hi - lo
        emb_c = np.zeros((Vmax, PDIMP), dtype=np.float32)
        emb_c[:span, :PDIM] = emb_table[lo:hi]
        rows = np.zeros(T * 128, dtype=np.int32)
        rows[: e - s] = (uniq[s:e] - lo).astype(np.int32)
        # int16 wrapped layout: index i -> [i % 16, i // 16], replicated x8
        wrapped = rows.astype(np.int16).reshape(T * 8, 16).T  # [16, T*8]
        idx16 = np.tile(wrapped, (8, 1)).copy()  # [128, T*8]
        selm = np.zeros((T * 128, B), dtype=np.float32)
        selm[: e - s] = cnt[s:e]
        in_maps.append(
            {
                "emb": emb_c,
                "idx16": idx16,
                "idx32": rows.reshape(T, 128).T.copy(),
                "sel": selm.reshape(T, 128, B).transpose(1, 0, 2).reshape(128, T * B).copy(),
                "wrep": wrep,
                "brep": brep,
            }
        )
    return T, Vmax, in_maps


def kernel(tokens, lengths, emb_table, W, b):
    from concourse.bass_utils import run_bass_kernel_spmd

    mode = os.environ.get("BERT_MODE", "dmag")
    chunk = int(os.environ.get("BERT_CHUNK", "8"))
    gbufs = int(os.environ.get("BERT_GBUFS", "3"))
    trace = os.environ.get("BERT_TRACE", "0") == "1"

    T, Vmax, in_maps = _marshal(tokens, lengths, emb_table, W, b)
    nc = _build(Vmax, T, chunk, gbufs, mode=mode)
    res = run_bass_kernel_spmd(nc, in_maps, core_ids=list(range(NCORES)), trace=trace)
    LAST["results"] = res
    LAST["T"] = T
    LAST["Vmax"] = Vmax
    return res.results[0]["out"].reshape(B).astype(np.float32)
